# revision 31
# baseline (speedup 1.0000x reference)
"""Trainium2 Bass kernel for nn_CustomLoss_35940286333129.

loss[b] = mean|pred-target| (mae, scalar)
        + mean(min_n cdist[b,n,m]) + mean(min_b cdist[b,n,m])  (chamfer, scalar)
        + mean|sort(pred[b].ravel()) - sort(target[b].ravel())|  (emd, per-b)

Sharding: data-parallel over batch B=32 across 8 NeuronCores (4 samples each).

Device kernel (per local sample b):
  One fp8 DoubleRow matmul per 128-row tile computes the COMPLETE squared
  distance d2[m, n] = tn[m] + pn[n] - 2*T[m].P[n] directly in PSUM:
  the K=256 contraction carries -2*T^t x P^t in the first K-half and the
  norm biases in the second K-half (tn/pn shipped from the host as 3-term
  fp8 residual cascades against ones rows). 512 PE cycles per tile;
  no ones-matmul, no cast/transpose chains, no DRAM bounce.

  One fused custom DVE op consumes each PSUM tile in a single 1x pass:
      out = where(Idx == 1023, running_min(d2), min(d2, acc))
  so cols 0..1022 update the cross-sample elementwise min (chamfer min over
  dim=0) while col 1023 captures min_n d2 (chamfer min over dim=1), which
  ACT harvests per sample before the next overwrite. PSUM holds four exact
  [128,1024] tiles (no pad column), double-buffering the PE four deep.

Host: fp8 operand packing (transpose/cast/norm cascades) during sharding,
cross-core elementwise min + sqrt + means, the exact column n=1023 of the
chamfer dim-0 min (overwritten on-device by the scan output; 32x1024 dot
products in numpy), mae, and the exact per-sample EMD via np.sort (sort is
unsupported on trn2).
"""

import numpy as np
import ml_dtypes

F8 = ml_dtypes.float8_e4m3

B, N, D = 32, 1024, 128
NCORES = 8
BL = B // NCORES          # 4 local samples per core
NT = N // 128             # 8 row tiles

_CACHE = {}


def _register_ops():
    from concourse import dve_ops
    from concourse.dve_ops import DveOp, OPS, DveOpSpec
    from concourse.dve_spec import (Spec, Src0, Src1, C0, C1, scan, minn,
                                    select, eq, lower, AluOp, Idx)

    def _mk(name, body, ref, rd1):
        for op in OPS:
            if op.name == name:
                return op
        spec = Spec(body=body, reference=ref)
        shas = {}
        for ver in ("v3", "v4"):
            tmp = DveOpSpec(name=name, opcode=0, uops=lower(spec, ver=ver),
                            rd1_en=rd1)
            shas[ver] = tmp.sha(ver)
        op = DveOp(name, spec, subdim=False, uops_sha=shas)
        OPS.append(op)
        dve_ops.CUSTOM_DVE_SPECS[op.name] = op.spec
        dve_ops._SUB_OPCODE_FOR_NAME[op.name] = (
            dve_ops._CUSTOM_DVE_ROW_BASE + len(OPS) - 1)
        return op

    r = scan(AluOp.MIN, Src0, init=C0)

    def ref_acc(in0, in1, s0, s1, imm2):
        idx = np.arange(in0.shape[-1])
        state = np.minimum.accumulate(np.minimum(in0, s0), axis=-1)
        return np.where(idx == s1, state, np.minimum(in0, in1))

    def ref_init(in0, s0, s1, imm2):
        idx = np.arange(in0.shape[-1])
        state = np.minimum.accumulate(np.minimum(in0, s0), axis=-1)
        return np.where(idx == s1, state, in0)

    acc_op = _mk("MINACC_IDX", select(eq(Idx, C1), r, minn(Src0, Src1)),
                 ref_acc, True)
    init_op = _mk("MININIT_IDX", select(eq(Idx, C1), r, Src0), ref_init, False)
    return acc_op, init_op, _register_2x_op()


def _build_2x_uops():
    """Hand-built 2X_1PORT program (HW-verified bit-exact vs the 1x body).

    Trigger-sequenced, no datapath counter: uop0 inits the scan flop S to
    MAX_POS; uops 1-3 stream 255+255+1=511 pairs computing WR0_LO =
    min(z_lo, a_lo), WR0_HI = min(z_hi, a_hi), S = min(S, z_lo, z_hi);
    uop4 takes the final pair with WR0_HI = min(S, z_lo, z_hi) — i.e. the
    row min lands at element 1023, matching the 1x body. Hardcodes a
    1024-element row. Chain c carries lane c+1 (lane 0 reachable only at
    block 0, left empty)."""
    from concourse.dve_uop import (UopConfig, UopDpConfig, InpSel, OutPath,
                                   OutSel, AluInp, DelayInp, Trigger, AluOp)

    INP = [InpSel.ZERO, InpSel.SRC_0, InpSel.SRC_1, InpSel.SRC_0_HI,
           InpSel.SRC_1_HI, InpSel.MAX_POS, InpSel.ZERO, InpSel.ZERO]
    INP_EN = [0, 1, 1, 1, 1, 1, 0, 0]

    def dp_block(op, s0, s1, cap4=False, cap5=False, aoe=1):
        delay = [DelayInp.PREV_DELAY] * 7
        if cap4:
            delay[4] = DelayInp.PREV_ALU_OUT
        if cap5:
            delay[5] = DelayInp.PREV_ALU_OUT
        return UopDpConfig(op=op, alu_src0=s0, alu_src1=s1, delay=delay,
                           alu_out_enable=aoe,
                           delay_enable=[1, 1, 1, 1, 1, 1, 0])

    def stream_blocks():
        return [
            dp_block(AluOp.MIN, AluInp.PREV_DELAY_0, AluInp.PREV_DELAY_1),
            dp_block(AluOp.MIN, AluInp.PREV_DELAY_2, AluInp.PREV_DELAY_3,
                     cap5=True),
            dp_block(AluOp.MIN, AluInp.PREV_DELAY_0, AluInp.PREV_DELAY_2,
                     cap4=True),
            dp_block(AluOp.MIN, AluInp.CURR_ALU_OUT, AluInp.PREV_ALU_OUT),
            dp_block(AluOp.BYPASS, AluInp.PREV_ALU_OUT, AluInp.PREV_ALU_OUT),
            dp_block(AluOp.BYPASS, AluInp.PREV_ALU_OUT, AluInp.PREV_ALU_OUT),
            dp_block(AluOp.BYPASS, AluInp.PREV_ALU_OUT, AluInp.PREV_ALU_OUT),
            dp_block(AluOp.BYPASS, AluInp.PREV_ALU_OUT, AluInp.PREV_ALU_OUT),
        ]

    def init_blocks():
        bp = lambda: dp_block(AluOp.BYPASS, AluInp.PREV_DELAY_0,
                              AluInp.PREV_DELAY_0, aoe=0)
        blocks = [bp(), bp(), bp(),
                  dp_block(AluOp.BYPASS, AluInp.PREV_DELAY_4,
                           AluInp.PREV_DELAY_4)]
        for _ in range(4):
            blocks.append(dp_block(AluOp.BYPASS, AluInp.PREV_ALU_OUT,
                                   AluInp.PREV_ALU_OUT, aoe=0))
        return blocks

    def mk(blocks, out, out_en, req, rep, trig, nxt):
        return UopConfig(inp=list(INP), inp_enable=list(INP_EN),
                         out=out, out_enable=out_en,
                         require_inp0=req, require_inp1=req,
                         repeat_count=rep, trigger=trig, next_uop=nxt,
                         datapath_config=blocks)

    OUT_OFF = {OutPath.WR0_LO: OutSel.ALU_OUT, OutPath.WR0_HI: OutSel.ALU_OUT,
               OutPath.WR1_LO: OutSel.ALU_OUT, OutPath.WR1_HI: OutSel.ALU_OUT}
    EN_OFF = {OutPath.WR0_LO: 0, OutPath.WR0_HI: 0,
              OutPath.WR1_LO: 0, OutPath.WR1_HI: 0}
    OUT_STREAM = {OutPath.WR0_LO: OutSel.DELAY_5,
                  OutPath.WR0_HI: OutSel.DELAY_4,
                  OutPath.WR1_LO: OutSel.ALU_OUT,
                  OutPath.WR1_HI: OutSel.ALU_OUT}
    EN_RW = {OutPath.WR0_LO: 1, OutPath.WR0_HI: 1,
             OutPath.WR1_LO: 0, OutPath.WR1_HI: 0}
    OUT_LAST = {OutPath.WR0_LO: OutSel.DELAY_5, OutPath.WR0_HI: OutSel.ALU_OUT,
                OutPath.WR1_LO: OutSel.ALU_OUT, OutPath.WR1_HI: OutSel.ALU_OUT}

    T = Trigger
    return [
        mk(init_blocks(), OUT_OFF, EN_OFF, 0, 1, (T.COUNT, T.NONE, T.NONE),
           (1, 0, 0)),
        mk(stream_blocks(), OUT_STREAM, EN_RW, 1, 255,
           (T.COUNT, T.NONE, T.NONE), (2, 0, 0)),
        mk(stream_blocks(), OUT_STREAM, EN_RW, 1, 255,
           (T.COUNT, T.NONE, T.NONE), (3, 0, 0)),
        mk(stream_blocks(), OUT_STREAM, EN_RW, 1, 1,
           (T.COUNT, T.NONE, T.NONE), (4, 0, 0)),
        mk(stream_blocks(), OUT_LAST, EN_RW, 1, 0,
           (T.SRC_TENSOR_DONE, T.NONE, T.NONE), (0, 0, 0)),
    ]


def _register_2x_op():
    from concourse import dve_ops
    from concourse.dve_ops import DveOp, OPS, DveOpSpec, get_dve_sub_opcode
    from concourse.dve_spec import (Spec, Src0, Src1, C0, C1, scan, minn,
                                    select, eq, lower, AluOp, Idx)

    for op in OPS:
        if op.name == "MINACC2X":
            return op

    r = scan(AluOp.MIN, Src0, init=C0)
    body = select(eq(Idx, C1), r, minn(Src0, Src1))

    def ref(in0, in1, s0, s1, imm2):
        idx = np.arange(in0.shape[-1])
        state = np.minimum.accumulate(np.minimum(in0, s0), axis=-1)
        return np.where(idx == s1, state, np.minimum(in0, in1))

    spec = Spec(body=body, reference=ref)

    class PerfDveOp(DveOp):
        def compile(self, ver):
            key = ("MINACC2X", ver)
            if key in dve_ops._COMPILE_CACHE:
                return dve_ops._COMPILE_CACHE[key]
            import copy
            uops_1x = lower(spec, ver=ver)
            while len(uops_1x) < 5:   # pad REGULAR to the 2x state count
                pad = copy.deepcopy(uops_1x[-1])
                pad.next_uop = (0, 0, 0)
                uops_1x.append(pad)
            result = DveOpSpec(
                name="MINACC2X", opcode=get_dve_sub_opcode("MINACC2X"),
                uops=uops_1x, uops_2x=_build_2x_uops(), rd1_en=True,
                perf_max=1)
            dve_ops._COMPILE_CACHE[key] = result
            return result

    op = PerfDveOp("MINACC2X", spec, subdim=False, uops_sha={})
    OPS.append(op)
    dve_ops.CUSTOM_DVE_SPECS[op.name] = op.spec
    dve_ops._SUB_OPCODE_FOR_NAME[op.name] = (
        dve_ops._CUSTOM_DVE_ROW_BASE + len(OPS) - 1)
    return op


def _emit_2x(nc, op, out, in0, in1):
    """InstCustomDveAnt with perf_max=1 (mirrors bass._custom_dve)."""
    from concourse import bass_isa, mybir
    from concourse.dve_ops import get_dve_sub_opcode
    v = nc.vector
    if op.name not in nc.m.ant_custom_dve_ops:
        nc.m.ant_custom_dve_ops = sorted({*nc.m.ant_custom_dve_ops, op.name})
    isa_opcode = nc.isa.Opcode[
        "NEURON_ISA_TPB_OPCODE_CUSTOM_DVE_ANT_"
        f"{bass_isa.CustomDveShape.TTSS.slot()}"].value
    ins = [v.lower_ap(in0, for_isa=True, opt=True),
           v.lower_ap(in1, for_isa=True, opt=True),
           mybir.ImmediateValue(dtype=mybir.dt.float32, value=60000.0),
           mybir.ImmediateValue(dtype=mybir.dt.float32, value=1023.0)]
    outs = [v.lower_ap(out, for_isa=True, opt=True)]
    return v.add_instruction(bass_isa.InstCustomDveAnt(
        name=nc.get_next_instruction_name(),
        op_name=op.name, rd1_en=True, subdim=0, imm2=0.0,
        shape=bass_isa.CustomDveShape.TTSS, row=get_dve_sub_opcode(op.name),
        isa_opcode=isa_opcode, ins=ins, outs=outs, perf_max=1))


def _build():
    import concourse.bass as bass
    import concourse.bacc as bacc
    import concourse.tile as tile
    from concourse import mybir

    MINACC, MININIT, MIN2X = _register_ops()
    NA = 2   # tiles consumed straight from PSUM by the 1x op per sample

    f32, f16, f8 = mybir.dt.float32, mybir.dt.float16, mybir.dt.float8e4
    AF = mybir.ActivationFunctionType
    DR = mybir.MatmulPerfMode.DoubleRow

    nc = bacc.Bacc("TRN2", target_bir_lowering=False, debug=False,
                   num_devices=NCORES)
    stat_d = nc.declare_dram_parameter("stat8", [BL, 128, NT, 2, 128], f8,
                                       isOutput=False)
    mov_d = nc.declare_dram_parameter("mov8", [BL, 128, 2, N], f8,
                                      isOutput=False)
    ch0_o = nc.declare_dram_parameter("ch0_part", [N, N], f16, isOutput=True)
    ch1_o = nc.declare_dram_parameter("ch1_part", [128, BL, NT], f16,
                                      isOutput=True)

    with tile.TileContext(nc) as tc:
        with (
            tc.tile_pool(name="stat", bufs=2) as statp,
            tc.tile_pool(name="mov", bufs=2) as movp,
            tc.tile_pool(name="z16p", bufs=6) as z16p,
            tc.tile_pool(name="persist", bufs=1) as perp,
            tc.tile_pool(name="nps", bufs=1, space=bass.MemorySpace.PSUM) as nps,
        ):
            acc = perp.tile([128, NT, N], f16, tag="acc")
            ch1z = perp.tile([128, BL, NT], f16, tag="ch1z")
            big = perp.tile([128, N], f16, tag="big")
            nc.gpsimd.memset(big[:], 60000.0)

            gt = [nps.tile([128, N], f32, tag=f"g{i}", name=f"g{i}")
                  for i in range(4)]

            for b in range(BL):
                # split loads so the first tiles' operands land early; b=0
                # fans out over three DMA queues to shorten the pipeline fill
                stat = statp.tile([128, NT, 2, 128], f8, tag="stat")
                mov = movp.tile([128, 2, N], f8, tag="mov")
                if b == 0:
                    nc.sync.dma_start(stat[:, 0:2], stat_d[b, :, 0:2])
                    nc.scalar.dma_start(mov[:, :, 0:512],
                                        mov_d[b, :, :, 0:512])
                    nc.gpsimd.dma_start(mov[:, :, 512:N],
                                        mov_d[b, :, :, 512:N])
                    nc.sync.dma_start(stat[:, 2:NT], stat_d[b, :, 2:NT])
                else:
                    nc.sync.dma_start(stat[:, 0:2], stat_d[b, :, 0:2])
                    nc.gpsimd.dma_start(mov[:, :, 0:512],
                                        mov_d[b, :, :, 0:512])
                    nc.sync.dma_start(stat[:, 2:NT], stat_d[b, :, 2:NT])
                    nc.gpsimd.dma_start(mov[:, :, 512:N],
                                        mov_d[b, :, :, 512:N])

                # A-tiles (0,1) consumed mid-stream so the ACT cast
                # pipeline builds inventory during the long 1x ops
                for seq, mt in enumerate((0, 2, 3, 1, 4, 5, 6, 7)):
                    g = gt[seq % 4]
                    for c in range(2):
                        nc.tensor.matmul(
                            g[:, c * 512:(c + 1) * 512],
                            stat[:, mt, :, :],
                            mov[:, :, c * 512:(c + 1) * 512],
                            start=True, stop=True, perf_mode=DR)
                    if mt < NA:
                        # PSUM-direct fused 1x consume
                        if b == 0:
                            nc.vector._custom_dve(
                                MININIT, out=acc[:, mt, :], in0=g[:],
                                s0=60000.0, s1=1023.0)
                        else:
                            nc.vector._custom_dve(
                                MINACC, out=acc[:, mt, :], in0=g[:],
                                in1=acc[:, mt, :], s0=60000.0, s1=1023.0)
                    else:
                        # ACT casts PSUM->fp16, then the 2X_1PORT op
                        z16 = z16p.tile([128, N], f16, tag="z16")
                        nc.scalar.activation(out=z16[:], in_=g[:],
                                             func=AF.Copy)
                        _emit_2x(nc, MIN2X, out=acc[:, mt, :], in0=z16[:],
                                 in1=(big[:] if b == 0
                                      else acc[:, mt, :]))
                    if b == BL - 1:
                        # acc[mt] final: stream it out under remaining
                        # compute, alternating rings; the last two tiles
                        # split so no single transfer tails past the end
                        lo, hi = mt * 128, (mt + 1) * 128
                        if mt < NT - 2:
                            ring = nc.gpsimd if mt % 2 == 0 else nc.sync
                            ring.dma_start(ch0_o[lo:hi, :], acc[:, mt, :])
                        else:
                            # scalar queue is idle by now; keep gpsimd free
                            # to drain its earlier tiles
                            nc.scalar.dma_start(ch0_o[lo:hi, 0:512],
                                                acc[:, mt, 0:512])
                            nc.sync.dma_start(ch0_o[lo:hi, 512:N],
                                              acc[:, mt, 512:N])
                # harvest this b's min_n d2 (scan cols) before b+1 overwrites;
                # two halves, so half 1 is done before b+1's first custom op
                nc.scalar.activation(out=ch1z[:, b, 0:4],
                                     in_=acc[:, 0:4, N - 1], func=AF.Copy)
                nc.scalar.activation(out=ch1z[:, b, 4:NT],
                                     in_=acc[:, 4:NT, N - 1], func=AF.Copy)
                nc.gpsimd.dma_start(ch1_o[:, b, :], ch1z[:, b, :])

    nc.compile()
    return nc


def _get_nc():
    if "nc" not in _CACHE:
        _CACHE["nc"] = _build()
    return _CACHE["nc"]


def _pack_core(pred_s, targ_s):
    """Build stat8/mov8 fp8 operands for one core's BL samples."""
    stat8 = np.zeros((BL, 128, NT, 2, 128), F8)
    mov8 = np.zeros((BL, 128, 2, N), F8)
    one8 = np.asarray(1.0, F8)
    for b in range(BL):
        T = targ_s[b]                    # [N, D]
        P = pred_s[b]
        tn = (T.astype(np.float64) ** 2).sum(-1).astype(np.float32)  # [N]
        pn = (P.astype(np.float64) ** 2).sum(-1).astype(np.float32)

        # 3-term fp8 residual cascades of tn / pn
        def casc(v):
            terms, rem = [], v.copy()
            for _ in range(3):
                t = np.asarray(rem, F8)
                terms.append(t)
                rem = rem - t.astype(np.float32)
            return terms

        tn_t, pn_t = casc(tn), casc(pn)

        Tt2 = np.asarray(-2.0 * T.T, F8)          # [d=128, m_global]
        stat8[b, :, :, 0, :] = Tt2.reshape(128, NT, 128)
        for j in range(3):
            stat8[b, j, :, 1, :] = one8                       # pn ones
            stat8[b, 3 + j, :, 1, :] = tn_t[j].reshape(NT, 128)

        mov8[b, :, 0, :] = np.asarray(P.T, F8)    # [d, n]
        for j in range(3):
            mov8[b, j, 1, :] = pn_t[j]
            mov8[b, 3 + j, 1, :] = one8
    return stat8, mov8


def run_device(pred, target, trace=False, **kw):
    from concourse.bass_utils import run_bass_kernel_spmd

    nc = _get_nc()
    ins = []
    for i in range(NCORES):
        sl = slice(i * BL, (i + 1) * BL)
        stat8, mov8 = _pack_core(pred[sl], target[sl])
        ins.append({"stat8": stat8, "mov8": mov8})
    return run_bass_kernel_spmd(nc, ins, list(range(NCORES)), trace=trace, **kw)


def kernel(pred, target):
    pred = np.ascontiguousarray(np.asarray(pred, dtype=np.float32))
    target = np.ascontiguousarray(np.asarray(target, dtype=np.float32))
    res = run_device(pred, target)
    rs = res.results

    # chamfer min over dim=0 (batch): cross-core elementwise min of acc
    d0 = rs[0]["ch0_part"].astype(np.float32)
    for r in rs[1:]:
        d0 = np.minimum(d0, r["ch0_part"].astype(np.float32))
    # col N-1 was overwritten by the scan output on device; recompute exact
    lastp = pred[:, N - 1, :]                              # [B, D]
    dlast = ((target.astype(np.float64)
              - lastp[:, None, :].astype(np.float64)) ** 2).sum(-1)  # [B, N]
    d0[:, N - 1] = dlast.min(axis=0)
    ch0 = np.sqrt(np.maximum(d0.astype(np.float64), 1e-12)).mean()

    # chamfer min over dim=1: scan cols, [core][p, b_local, mt] -> [B, N]
    ch1 = np.concatenate(
        [r["ch1_part"].astype(np.float64).transpose(1, 2, 0).reshape(BL, N)
         for r in rs], axis=0)                              # [B, N]
    ch1 = np.sqrt(np.maximum(ch1, 1e-12)).mean()

    mae = np.abs(pred.astype(np.float64) - target.astype(np.float64)).mean()

    p = np.sort(pred.reshape(B, -1), axis=1)
    g = np.sort(target.reshape(B, -1), axis=1)
    emd = np.abs(p - g).mean(axis=1, dtype=np.float64)

    return (mae + ch0 + ch1 + emd).astype(np.float32)


# revision 32
# speedup vs baseline: 1.0250x; 1.0250x over previous
"""Trainium2 Bass kernel for nn_CustomLoss_35940286333129.

loss[b] = mean|pred-target| (mae, scalar)
        + mean(min_n cdist[b,n,m]) + mean(min_b cdist[b,n,m])  (chamfer, scalar)
        + mean|sort(pred[b].ravel()) - sort(target[b].ravel())|  (emd, per-b)

Sharding: data-parallel over batch B=32 across 8 NeuronCores (4 samples each).

Device kernel (per local sample b):
  One fp8 DoubleRow matmul per 128-row tile computes the COMPLETE squared
  distance d2[m, n] = tn[m] + pn[n] - 2*T[m].P[n] directly in PSUM:
  the K=256 contraction carries -2*T^t x P^t in the first K-half and the
  norm biases in the second K-half (tn/pn shipped from the host as 3-term
  fp8 residual cascades against ones rows). 512 PE cycles per tile;
  no ones-matmul, no cast/transpose chains, no DRAM bounce.

  One fused custom DVE op consumes each PSUM tile in a single 1x pass:
      out = where(Idx == 1023, running_min(d2), min(d2, acc))
  so cols 0..1022 update the cross-sample elementwise min (chamfer min over
  dim=0) while col 1023 captures min_n d2 (chamfer min over dim=1), which
  ACT harvests per sample before the next overwrite. PSUM holds four exact
  [128,1024] tiles (no pad column), double-buffering the PE four deep.

Host: fp8 operand packing (transpose/cast/norm cascades) during sharding,
cross-core elementwise min + sqrt + means, the exact column n=1023 of the
chamfer dim-0 min (overwritten on-device by the scan output; 32x1024 dot
products in numpy), mae, and the exact per-sample EMD via np.sort (sort is
unsupported on trn2).
"""

import numpy as np
import ml_dtypes

F8 = ml_dtypes.float8_e4m3

B, N, D = 32, 1024, 128
NCORES = 8
BL = B // NCORES          # 4 local samples per core
NT = N // 128             # 8 row tiles

_CACHE = {}


def _register_ops():
    from concourse import dve_ops
    from concourse.dve_ops import DveOp, OPS, DveOpSpec
    from concourse.dve_spec import (Spec, Src0, Src1, C0, C1, scan, minn,
                                    select, eq, lower, AluOp, Idx)

    def _mk(name, body, ref, rd1):
        for op in OPS:
            if op.name == name:
                return op
        spec = Spec(body=body, reference=ref)
        shas = {}
        for ver in ("v3", "v4"):
            tmp = DveOpSpec(name=name, opcode=0, uops=lower(spec, ver=ver),
                            rd1_en=rd1)
            shas[ver] = tmp.sha(ver)
        op = DveOp(name, spec, subdim=False, uops_sha=shas)
        OPS.append(op)
        dve_ops.CUSTOM_DVE_SPECS[op.name] = op.spec
        dve_ops._SUB_OPCODE_FOR_NAME[op.name] = (
            dve_ops._CUSTOM_DVE_ROW_BASE + len(OPS) - 1)
        return op

    r = scan(AluOp.MIN, Src0, init=C0)

    def ref_acc(in0, in1, s0, s1, imm2):
        idx = np.arange(in0.shape[-1])
        state = np.minimum.accumulate(np.minimum(in0, s0), axis=-1)
        return np.where(idx == s1, state, np.minimum(in0, in1))

    def ref_init(in0, s0, s1, imm2):
        idx = np.arange(in0.shape[-1])
        state = np.minimum.accumulate(np.minimum(in0, s0), axis=-1)
        return np.where(idx == s1, state, in0)

    acc_op = _mk("MINACC_IDX", select(eq(Idx, C1), r, minn(Src0, Src1)),
                 ref_acc, True)
    init_op = _mk("MININIT_IDX", select(eq(Idx, C1), r, Src0), ref_init, False)
    return acc_op, init_op, _register_2x_op()


def _build_2x_uops():
    """Hand-built 2X_1PORT program (HW-verified bit-exact vs the 1x body).

    Trigger-sequenced, no datapath counter: uop0 inits the scan flop S to
    MAX_POS; uops 1-3 stream 255+255+1=511 pairs computing WR0_LO =
    min(z_lo, a_lo), WR0_HI = min(z_hi, a_hi), S = min(S, z_lo, z_hi);
    uop4 takes the final pair with WR0_HI = min(S, z_lo, z_hi) — i.e. the
    row min lands at element 1023, matching the 1x body. Hardcodes a
    1024-element row. Chain c carries lane c+1 (lane 0 reachable only at
    block 0, left empty)."""
    from concourse.dve_uop import (UopConfig, UopDpConfig, InpSel, OutPath,
                                   OutSel, AluInp, DelayInp, Trigger, AluOp)

    INP = [InpSel.ZERO, InpSel.SRC_0, InpSel.SRC_1, InpSel.SRC_0_HI,
           InpSel.SRC_1_HI, InpSel.MAX_POS, InpSel.ZERO, InpSel.ZERO]
    INP_EN = [0, 1, 1, 1, 1, 1, 0, 0]

    def dp_block(op, s0, s1, cap4=False, cap5=False, aoe=1):
        delay = [DelayInp.PREV_DELAY] * 7
        if cap4:
            delay[4] = DelayInp.PREV_ALU_OUT
        if cap5:
            delay[5] = DelayInp.PREV_ALU_OUT
        return UopDpConfig(op=op, alu_src0=s0, alu_src1=s1, delay=delay,
                           alu_out_enable=aoe,
                           delay_enable=[1, 1, 1, 1, 1, 1, 0])

    def stream_blocks():
        return [
            dp_block(AluOp.MIN, AluInp.PREV_DELAY_0, AluInp.PREV_DELAY_1),
            dp_block(AluOp.MIN, AluInp.PREV_DELAY_2, AluInp.PREV_DELAY_3,
                     cap5=True),
            dp_block(AluOp.MIN, AluInp.PREV_DELAY_0, AluInp.PREV_DELAY_2,
                     cap4=True),
            dp_block(AluOp.MIN, AluInp.CURR_ALU_OUT, AluInp.PREV_ALU_OUT),
            dp_block(AluOp.BYPASS, AluInp.PREV_ALU_OUT, AluInp.PREV_ALU_OUT),
            dp_block(AluOp.BYPASS, AluInp.PREV_ALU_OUT, AluInp.PREV_ALU_OUT),
            dp_block(AluOp.BYPASS, AluInp.PREV_ALU_OUT, AluInp.PREV_ALU_OUT),
            dp_block(AluOp.BYPASS, AluInp.PREV_ALU_OUT, AluInp.PREV_ALU_OUT),
        ]

    def init_blocks():
        bp = lambda: dp_block(AluOp.BYPASS, AluInp.PREV_DELAY_0,
                              AluInp.PREV_DELAY_0, aoe=0)
        blocks = [bp(), bp(), bp(),
                  dp_block(AluOp.BYPASS, AluInp.PREV_DELAY_4,
                           AluInp.PREV_DELAY_4)]
        for _ in range(4):
            blocks.append(dp_block(AluOp.BYPASS, AluInp.PREV_ALU_OUT,
                                   AluInp.PREV_ALU_OUT, aoe=0))
        return blocks

    def mk(blocks, out, out_en, req, rep, trig, nxt):
        return UopConfig(inp=list(INP), inp_enable=list(INP_EN),
                         out=out, out_enable=out_en,
                         require_inp0=req, require_inp1=req,
                         repeat_count=rep, trigger=trig, next_uop=nxt,
                         datapath_config=blocks)

    OUT_OFF = {OutPath.WR0_LO: OutSel.ALU_OUT, OutPath.WR0_HI: OutSel.ALU_OUT,
               OutPath.WR1_LO: OutSel.ALU_OUT, OutPath.WR1_HI: OutSel.ALU_OUT}
    EN_OFF = {OutPath.WR0_LO: 0, OutPath.WR0_HI: 0,
              OutPath.WR1_LO: 0, OutPath.WR1_HI: 0}
    OUT_STREAM = {OutPath.WR0_LO: OutSel.DELAY_5,
                  OutPath.WR0_HI: OutSel.DELAY_4,
                  OutPath.WR1_LO: OutSel.ALU_OUT,
                  OutPath.WR1_HI: OutSel.ALU_OUT}
    EN_RW = {OutPath.WR0_LO: 1, OutPath.WR0_HI: 1,
             OutPath.WR1_LO: 0, OutPath.WR1_HI: 0}
    OUT_LAST = {OutPath.WR0_LO: OutSel.DELAY_5, OutPath.WR0_HI: OutSel.ALU_OUT,
                OutPath.WR1_LO: OutSel.ALU_OUT, OutPath.WR1_HI: OutSel.ALU_OUT}

    T = Trigger
    return [
        mk(init_blocks(), OUT_OFF, EN_OFF, 0, 1, (T.COUNT, T.NONE, T.NONE),
           (1, 0, 0)),
        mk(stream_blocks(), OUT_STREAM, EN_RW, 1, 255,
           (T.COUNT, T.NONE, T.NONE), (2, 0, 0)),
        mk(stream_blocks(), OUT_STREAM, EN_RW, 1, 255,
           (T.COUNT, T.NONE, T.NONE), (3, 0, 0)),
        mk(stream_blocks(), OUT_STREAM, EN_RW, 1, 1,
           (T.COUNT, T.NONE, T.NONE), (4, 0, 0)),
        mk(stream_blocks(), OUT_LAST, EN_RW, 1, 0,
           (T.SRC_TENSOR_DONE, T.NONE, T.NONE), (0, 0, 0)),
    ]


def _register_2x_op():
    from concourse import dve_ops
    from concourse.dve_ops import DveOp, OPS, DveOpSpec, get_dve_sub_opcode
    from concourse.dve_spec import (Spec, Src0, Src1, C0, C1, scan, minn,
                                    select, eq, lower, AluOp, Idx)

    for op in OPS:
        if op.name == "MINACC2X":
            return op

    r = scan(AluOp.MIN, Src0, init=C0)
    body = select(eq(Idx, C1), r, minn(Src0, Src1))

    def ref(in0, in1, s0, s1, imm2):
        idx = np.arange(in0.shape[-1])
        state = np.minimum.accumulate(np.minimum(in0, s0), axis=-1)
        return np.where(idx == s1, state, np.minimum(in0, in1))

    spec = Spec(body=body, reference=ref)

    class PerfDveOp(DveOp):
        def compile(self, ver):
            key = ("MINACC2X", ver)
            if key in dve_ops._COMPILE_CACHE:
                return dve_ops._COMPILE_CACHE[key]
            import copy
            uops_1x = lower(spec, ver=ver)
            while len(uops_1x) < 5:   # pad REGULAR to the 2x state count
                pad = copy.deepcopy(uops_1x[-1])
                pad.next_uop = (0, 0, 0)
                uops_1x.append(pad)
            result = DveOpSpec(
                name="MINACC2X", opcode=get_dve_sub_opcode("MINACC2X"),
                uops=uops_1x, uops_2x=_build_2x_uops(), rd1_en=True,
                perf_max=1)
            dve_ops._COMPILE_CACHE[key] = result
            return result

    op = PerfDveOp("MINACC2X", spec, subdim=False, uops_sha={})
    OPS.append(op)
    dve_ops.CUSTOM_DVE_SPECS[op.name] = op.spec
    dve_ops._SUB_OPCODE_FOR_NAME[op.name] = (
        dve_ops._CUSTOM_DVE_ROW_BASE + len(OPS) - 1)
    return op


def _emit_2x(nc, op, out, in0, in1):
    """InstCustomDveAnt with perf_max=1 (mirrors bass._custom_dve)."""
    from concourse import bass_isa, mybir
    from concourse.dve_ops import get_dve_sub_opcode
    v = nc.vector
    if op.name not in nc.m.ant_custom_dve_ops:
        nc.m.ant_custom_dve_ops = sorted({*nc.m.ant_custom_dve_ops, op.name})
    isa_opcode = nc.isa.Opcode[
        "NEURON_ISA_TPB_OPCODE_CUSTOM_DVE_ANT_"
        f"{bass_isa.CustomDveShape.TTSS.slot()}"].value
    ins = [v.lower_ap(in0, for_isa=True, opt=True),
           v.lower_ap(in1, for_isa=True, opt=True),
           mybir.ImmediateValue(dtype=mybir.dt.float32, value=60000.0),
           mybir.ImmediateValue(dtype=mybir.dt.float32, value=1023.0)]
    outs = [v.lower_ap(out, for_isa=True, opt=True)]
    return v.add_instruction(bass_isa.InstCustomDveAnt(
        name=nc.get_next_instruction_name(),
        op_name=op.name, rd1_en=True, subdim=0, imm2=0.0,
        shape=bass_isa.CustomDveShape.TTSS, row=get_dve_sub_opcode(op.name),
        isa_opcode=isa_opcode, ins=ins, outs=outs, perf_max=1))


def _build():
    import concourse.bass as bass
    import concourse.bacc as bacc
    import concourse.tile as tile
    from concourse import mybir

    MINACC, MININIT, MIN2X = _register_ops()
    NA = 2   # tiles consumed straight from PSUM by the 1x op per sample

    f32, f16, f8 = mybir.dt.float32, mybir.dt.float16, mybir.dt.float8e4
    AF = mybir.ActivationFunctionType
    DR = mybir.MatmulPerfMode.DoubleRow

    nc = bacc.Bacc("TRN2", target_bir_lowering=False, debug=False,
                   num_devices=NCORES)
    stat_d = nc.declare_dram_parameter("stat8", [BL, 128, NT, 2, 128], f8,
                                       isOutput=False)
    mov_d = nc.declare_dram_parameter("mov8", [BL, 128, 2, N], f8,
                                      isOutput=False)
    ch0_o = nc.declare_dram_parameter("ch0_part", [N, N], f16, isOutput=True)
    ch1_o = nc.declare_dram_parameter("ch1_part", [128, BL, NT], f16,
                                      isOutput=True)

    with tile.TileContext(nc) as tc:
        with (
            tc.tile_pool(name="stat", bufs=2) as statp,
            tc.tile_pool(name="mov", bufs=2) as movp,
            tc.tile_pool(name="z16p", bufs=4) as z16p,
            tc.tile_pool(name="persist", bufs=1) as perp,
            tc.tile_pool(name="nps", bufs=1, space=bass.MemorySpace.PSUM) as nps,
        ):
            acc = perp.tile([128, NT, N], f16, tag="acc")
            ch1z = perp.tile([128, BL, NT], f16, tag="ch1z")
            big = perp.tile([128, N], f16, tag="big")
            nc.gpsimd.memset(big[:], 60000.0)

            gt = [nps.tile([128, N], f32, tag=f"g{i}", name=f"g{i}")
                  for i in range(4)]

            for b in range(BL):
                # split loads so the first tiles' operands land early; b=0
                # fans out over three DMA queues to shorten the pipeline fill
                stat = statp.tile([128, NT, 2, 128], f8, tag="stat")
                mov = movp.tile([128, 2, N], f8, tag="mov")
                if b == 0:
                    nc.sync.dma_start(stat[:, 0:2], stat_d[b, :, 0:2])
                    nc.scalar.dma_start(mov[:, :, 0:512],
                                        mov_d[b, :, :, 0:512])
                    nc.gpsimd.dma_start(mov[:, :, 512:N],
                                        mov_d[b, :, :, 512:N])
                    nc.sync.dma_start(stat[:, 2:NT], stat_d[b, :, 2:NT])
                else:
                    nc.sync.dma_start(stat[:, 0:2], stat_d[b, :, 0:2])
                    nc.gpsimd.dma_start(mov[:, :, 0:512],
                                        mov_d[b, :, :, 0:512])
                    nc.sync.dma_start(stat[:, 2:NT], stat_d[b, :, 2:NT])
                    nc.gpsimd.dma_start(mov[:, :, 512:N],
                                        mov_d[b, :, :, 512:N])

                # A-tiles (0,1) consumed mid-stream so the ACT cast
                # pipeline builds inventory during the long 1x ops
                for seq, mt in enumerate((0, 2, 3, 1, 4, 5, 6, 7)):
                    g = gt[seq % 4]
                    for c in range(2):
                        nc.tensor.matmul(
                            g[:, c * 512:(c + 1) * 512],
                            stat[:, mt, :, :],
                            mov[:, :, c * 512:(c + 1) * 512],
                            start=True, stop=True, perf_mode=DR)
                    if mt < NA:
                        # PSUM-direct fused 1x consume
                        if b == 0:
                            nc.vector._custom_dve(
                                MININIT, out=acc[:, mt, :], in0=g[:],
                                s0=60000.0, s1=1023.0)
                        else:
                            nc.vector._custom_dve(
                                MINACC, out=acc[:, mt, :], in0=g[:],
                                in1=acc[:, mt, :], s0=60000.0, s1=1023.0)
                    else:
                        # ACT casts PSUM->fp16, then the 2X_1PORT op
                        z16 = z16p.tile([128, N], f16, tag="z16")
                        nc.scalar.activation(out=z16[:], in_=g[:],
                                             func=AF.Copy)
                        _emit_2x(nc, MIN2X, out=acc[:, mt, :], in0=z16[:],
                                 in1=(big[:] if b == 0
                                      else acc[:, mt, :]))
                    if b == BL - 1:
                        # acc[mt] final: stream it out under remaining
                        # compute, alternating rings; the last two tiles
                        # split so no single transfer tails past the end
                        lo, hi = mt * 128, (mt + 1) * 128
                        if mt < NT - 2:
                            ring = nc.gpsimd if mt % 2 == 0 else nc.sync
                            ring.dma_start(ch0_o[lo:hi, :], acc[:, mt, :])
                        else:
                            # scalar queue is idle by now; keep gpsimd free
                            # to drain its earlier tiles
                            nc.scalar.dma_start(ch0_o[lo:hi, 0:512],
                                                acc[:, mt, 0:512])
                            nc.sync.dma_start(ch0_o[lo:hi, 512:N],
                                              acc[:, mt, 512:N])
                # harvest this b's min_n d2 (scan cols) before b+1 overwrites;
                # two halves, so half 1 is done before b+1's first custom op
                nc.scalar.activation(out=ch1z[:, b, 0:4],
                                     in_=acc[:, 0:4, N - 1], func=AF.Copy)
                nc.scalar.activation(out=ch1z[:, b, 4:NT],
                                     in_=acc[:, 4:NT, N - 1], func=AF.Copy)
                nc.gpsimd.dma_start(ch1_o[:, b, :], ch1z[:, b, :])

    nc.compile()
    return nc


def _get_nc():
    if "nc" not in _CACHE:
        _CACHE["nc"] = _build()
    return _CACHE["nc"]


def _pack_core(pred_s, targ_s):
    """Build stat8/mov8 fp8 operands for one core's BL samples."""
    stat8 = np.zeros((BL, 128, NT, 2, 128), F8)
    mov8 = np.zeros((BL, 128, 2, N), F8)
    one8 = np.asarray(1.0, F8)
    for b in range(BL):
        T = targ_s[b]                    # [N, D]
        P = pred_s[b]
        tn = (T.astype(np.float64) ** 2).sum(-1).astype(np.float32)  # [N]
        pn = (P.astype(np.float64) ** 2).sum(-1).astype(np.float32)

        # 3-term fp8 residual cascades of tn / pn
        def casc(v):
            terms, rem = [], v.copy()
            for _ in range(3):
                t = np.asarray(rem, F8)
                terms.append(t)
                rem = rem - t.astype(np.float32)
            return terms

        tn_t, pn_t = casc(tn), casc(pn)

        Tt2 = np.asarray(-2.0 * T.T, F8)          # [d=128, m_global]
        stat8[b, :, :, 0, :] = Tt2.reshape(128, NT, 128)
        for j in range(3):
            stat8[b, j, :, 1, :] = one8                       # pn ones
            stat8[b, 3 + j, :, 1, :] = tn_t[j].reshape(NT, 128)

        mov8[b, :, 0, :] = np.asarray(P.T, F8)    # [d, n]
        for j in range(3):
            mov8[b, j, 1, :] = pn_t[j]
            mov8[b, 3 + j, 1, :] = one8
    return stat8, mov8


def run_device(pred, target, trace=False, **kw):
    from concourse.bass_utils import run_bass_kernel_spmd

    nc = _get_nc()
    ins = []
    for i in range(NCORES):
        sl = slice(i * BL, (i + 1) * BL)
        stat8, mov8 = _pack_core(pred[sl], target[sl])
        ins.append({"stat8": stat8, "mov8": mov8})
    return run_bass_kernel_spmd(nc, ins, list(range(NCORES)), trace=trace, **kw)


def kernel(pred, target):
    pred = np.ascontiguousarray(np.asarray(pred, dtype=np.float32))
    target = np.ascontiguousarray(np.asarray(target, dtype=np.float32))
    res = run_device(pred, target)
    rs = res.results

    # chamfer min over dim=0 (batch): cross-core elementwise min of acc
    d0 = rs[0]["ch0_part"].astype(np.float32)
    for r in rs[1:]:
        d0 = np.minimum(d0, r["ch0_part"].astype(np.float32))
    # col N-1 was overwritten by the scan output on device; recompute exact
    lastp = pred[:, N - 1, :]                              # [B, D]
    dlast = ((target.astype(np.float64)
              - lastp[:, None, :].astype(np.float64)) ** 2).sum(-1)  # [B, N]
    d0[:, N - 1] = dlast.min(axis=0)
    ch0 = np.sqrt(np.maximum(d0.astype(np.float64), 1e-12)).mean()

    # chamfer min over dim=1: scan cols, [core][p, b_local, mt] -> [B, N]
    ch1 = np.concatenate(
        [r["ch1_part"].astype(np.float64).transpose(1, 2, 0).reshape(BL, N)
         for r in rs], axis=0)                              # [B, N]
    ch1 = np.sqrt(np.maximum(ch1, 1e-12)).mean()

    mae = np.abs(pred.astype(np.float64) - target.astype(np.float64)).mean()

    p = np.sort(pred.reshape(B, -1), axis=1)
    g = np.sort(target.reshape(B, -1), axis=1)
    emd = np.abs(p - g).mean(axis=1, dtype=np.float64)

    return (mae + ch0 + ch1 + emd).astype(np.float32)


# revision 33
# speedup vs baseline: 1.0473x; 1.0217x over previous
"""Trainium2 Bass kernel for nn_CustomLoss_35940286333129.

loss[b] = mean|pred-target| (mae, scalar)
        + mean(min_n cdist[b,n,m]) + mean(min_b cdist[b,n,m])  (chamfer, scalar)
        + mean|sort(pred[b].ravel()) - sort(target[b].ravel())|  (emd, per-b)

Sharding: data-parallel over batch B=32 across 8 NeuronCores (4 samples each).

Device kernel (per local sample b):
  One fp8 DoubleRow matmul per 128-row tile computes the COMPLETE squared
  distance d2[m, n] = tn[m] + pn[n] - 2*T[m].P[n] directly in PSUM:
  the K=256 contraction carries -2*T^t x P^t in the first K-half and the
  norm biases in the second K-half (tn/pn shipped from the host as 3-term
  fp8 residual cascades against ones rows). 512 PE cycles per tile;
  no ones-matmul, no cast/transpose chains, no DRAM bounce.

  One fused custom DVE op consumes each PSUM tile in a single 1x pass:
      out = where(Idx == 1023, running_min(d2), min(d2, acc))
  so cols 0..1022 update the cross-sample elementwise min (chamfer min over
  dim=0) while col 1023 captures min_n d2 (chamfer min over dim=1), which
  ACT harvests per sample before the next overwrite. PSUM holds four exact
  [128,1024] tiles (no pad column), double-buffering the PE four deep.

Host: fp8 operand packing (transpose/cast/norm cascades) during sharding,
cross-core elementwise min + sqrt + means, the exact column n=1023 of the
chamfer dim-0 min (overwritten on-device by the scan output; 32x1024 dot
products in numpy), mae, and the exact per-sample EMD via np.sort (sort is
unsupported on trn2).
"""

import numpy as np
import ml_dtypes

F8 = ml_dtypes.float8_e4m3

B, N, D = 32, 1024, 128
NCORES = 8
BL = B // NCORES          # 4 local samples per core
NT = N // 128             # 8 row tiles

_CACHE = {}


def _register_ops():
    from concourse import dve_ops
    from concourse.dve_ops import DveOp, OPS, DveOpSpec
    from concourse.dve_spec import (Spec, Src0, Src1, C0, C1, scan, minn,
                                    select, eq, lower, AluOp, Idx)

    def _mk(name, body, ref, rd1):
        for op in OPS:
            if op.name == name:
                return op
        spec = Spec(body=body, reference=ref)
        shas = {}
        for ver in ("v3", "v4"):
            tmp = DveOpSpec(name=name, opcode=0, uops=lower(spec, ver=ver),
                            rd1_en=rd1)
            shas[ver] = tmp.sha(ver)
        op = DveOp(name, spec, subdim=False, uops_sha=shas)
        OPS.append(op)
        dve_ops.CUSTOM_DVE_SPECS[op.name] = op.spec
        dve_ops._SUB_OPCODE_FOR_NAME[op.name] = (
            dve_ops._CUSTOM_DVE_ROW_BASE + len(OPS) - 1)
        return op

    r = scan(AluOp.MIN, Src0, init=C0)

    def ref_acc(in0, in1, s0, s1, imm2):
        idx = np.arange(in0.shape[-1])
        state = np.minimum.accumulate(np.minimum(in0, s0), axis=-1)
        return np.where(idx == s1, state, np.minimum(in0, in1))

    def ref_init(in0, s0, s1, imm2):
        idx = np.arange(in0.shape[-1])
        state = np.minimum.accumulate(np.minimum(in0, s0), axis=-1)
        return np.where(idx == s1, state, in0)

    acc_op = _mk("MINACC_IDX", select(eq(Idx, C1), r, minn(Src0, Src1)),
                 ref_acc, True)
    init_op = _mk("MININIT_IDX", select(eq(Idx, C1), r, Src0), ref_init, False)
    return acc_op, init_op, _register_2x_op()


def _build_2x_uops():
    """Hand-built 2X_1PORT program (HW-verified bit-exact vs the 1x body).

    Trigger-sequenced, no datapath counter: uop0 inits the scan flop S to
    MAX_POS; uops 1-3 stream 255+255+1=511 pairs computing WR0_LO =
    min(z_lo, a_lo), WR0_HI = min(z_hi, a_hi), S = min(S, z_lo, z_hi);
    uop4 takes the final pair with WR0_HI = min(S, z_lo, z_hi) — i.e. the
    row min lands at element 1023, matching the 1x body. Hardcodes a
    1024-element row. Chain c carries lane c+1 (lane 0 reachable only at
    block 0, left empty)."""
    from concourse.dve_uop import (UopConfig, UopDpConfig, InpSel, OutPath,
                                   OutSel, AluInp, DelayInp, Trigger, AluOp)

    INP = [InpSel.ZERO, InpSel.SRC_0, InpSel.SRC_1, InpSel.SRC_0_HI,
           InpSel.SRC_1_HI, InpSel.MAX_POS, InpSel.ZERO, InpSel.ZERO]
    INP_EN = [0, 1, 1, 1, 1, 1, 0, 0]

    def dp_block(op, s0, s1, cap4=False, cap5=False, aoe=1):
        delay = [DelayInp.PREV_DELAY] * 7
        if cap4:
            delay[4] = DelayInp.PREV_ALU_OUT
        if cap5:
            delay[5] = DelayInp.PREV_ALU_OUT
        return UopDpConfig(op=op, alu_src0=s0, alu_src1=s1, delay=delay,
                           alu_out_enable=aoe,
                           delay_enable=[1, 1, 1, 1, 1, 1, 0])

    def stream_blocks():
        return [
            dp_block(AluOp.MIN, AluInp.PREV_DELAY_0, AluInp.PREV_DELAY_1),
            dp_block(AluOp.MIN, AluInp.PREV_DELAY_2, AluInp.PREV_DELAY_3,
                     cap5=True),
            dp_block(AluOp.MIN, AluInp.PREV_DELAY_0, AluInp.PREV_DELAY_2,
                     cap4=True),
            dp_block(AluOp.MIN, AluInp.CURR_ALU_OUT, AluInp.PREV_ALU_OUT),
            dp_block(AluOp.BYPASS, AluInp.PREV_ALU_OUT, AluInp.PREV_ALU_OUT),
            dp_block(AluOp.BYPASS, AluInp.PREV_ALU_OUT, AluInp.PREV_ALU_OUT),
            dp_block(AluOp.BYPASS, AluInp.PREV_ALU_OUT, AluInp.PREV_ALU_OUT),
            dp_block(AluOp.BYPASS, AluInp.PREV_ALU_OUT, AluInp.PREV_ALU_OUT),
        ]

    def init_blocks():
        bp = lambda: dp_block(AluOp.BYPASS, AluInp.PREV_DELAY_0,
                              AluInp.PREV_DELAY_0, aoe=0)
        blocks = [bp(), bp(), bp(),
                  dp_block(AluOp.BYPASS, AluInp.PREV_DELAY_4,
                           AluInp.PREV_DELAY_4)]
        for _ in range(4):
            blocks.append(dp_block(AluOp.BYPASS, AluInp.PREV_ALU_OUT,
                                   AluInp.PREV_ALU_OUT, aoe=0))
        return blocks

    def mk(blocks, out, out_en, req, rep, trig, nxt):
        return UopConfig(inp=list(INP), inp_enable=list(INP_EN),
                         out=out, out_enable=out_en,
                         require_inp0=req, require_inp1=req,
                         repeat_count=rep, trigger=trig, next_uop=nxt,
                         datapath_config=blocks)

    OUT_OFF = {OutPath.WR0_LO: OutSel.ALU_OUT, OutPath.WR0_HI: OutSel.ALU_OUT,
               OutPath.WR1_LO: OutSel.ALU_OUT, OutPath.WR1_HI: OutSel.ALU_OUT}
    EN_OFF = {OutPath.WR0_LO: 0, OutPath.WR0_HI: 0,
              OutPath.WR1_LO: 0, OutPath.WR1_HI: 0}
    OUT_STREAM = {OutPath.WR0_LO: OutSel.DELAY_5,
                  OutPath.WR0_HI: OutSel.DELAY_4,
                  OutPath.WR1_LO: OutSel.ALU_OUT,
                  OutPath.WR1_HI: OutSel.ALU_OUT}
    EN_RW = {OutPath.WR0_LO: 1, OutPath.WR0_HI: 1,
             OutPath.WR1_LO: 0, OutPath.WR1_HI: 0}
    OUT_LAST = {OutPath.WR0_LO: OutSel.DELAY_5, OutPath.WR0_HI: OutSel.ALU_OUT,
                OutPath.WR1_LO: OutSel.ALU_OUT, OutPath.WR1_HI: OutSel.ALU_OUT}

    T = Trigger
    return [
        mk(init_blocks(), OUT_OFF, EN_OFF, 0, 1, (T.COUNT, T.NONE, T.NONE),
           (1, 0, 0)),
        mk(stream_blocks(), OUT_STREAM, EN_RW, 1, 255,
           (T.COUNT, T.NONE, T.NONE), (2, 0, 0)),
        mk(stream_blocks(), OUT_STREAM, EN_RW, 1, 255,
           (T.COUNT, T.NONE, T.NONE), (3, 0, 0)),
        mk(stream_blocks(), OUT_STREAM, EN_RW, 1, 1,
           (T.COUNT, T.NONE, T.NONE), (4, 0, 0)),
        mk(stream_blocks(), OUT_LAST, EN_RW, 1, 0,
           (T.SRC_TENSOR_DONE, T.NONE, T.NONE), (0, 0, 0)),
    ]


def _register_2x_op():
    from concourse import dve_ops
    from concourse.dve_ops import DveOp, OPS, DveOpSpec, get_dve_sub_opcode
    from concourse.dve_spec import (Spec, Src0, Src1, C0, C1, scan, minn,
                                    select, eq, lower, AluOp, Idx)

    for op in OPS:
        if op.name == "MINACC2X":
            return op

    r = scan(AluOp.MIN, Src0, init=C0)
    body = select(eq(Idx, C1), r, minn(Src0, Src1))

    def ref(in0, in1, s0, s1, imm2):
        idx = np.arange(in0.shape[-1])
        state = np.minimum.accumulate(np.minimum(in0, s0), axis=-1)
        return np.where(idx == s1, state, np.minimum(in0, in1))

    spec = Spec(body=body, reference=ref)

    class PerfDveOp(DveOp):
        def compile(self, ver):
            key = ("MINACC2X", ver)
            if key in dve_ops._COMPILE_CACHE:
                return dve_ops._COMPILE_CACHE[key]
            import copy
            uops_1x = lower(spec, ver=ver)
            while len(uops_1x) < 5:   # pad REGULAR to the 2x state count
                pad = copy.deepcopy(uops_1x[-1])
                pad.next_uop = (0, 0, 0)
                uops_1x.append(pad)
            result = DveOpSpec(
                name="MINACC2X", opcode=get_dve_sub_opcode("MINACC2X"),
                uops=uops_1x, uops_2x=_build_2x_uops(), rd1_en=True,
                perf_max=1)
            dve_ops._COMPILE_CACHE[key] = result
            return result

    op = PerfDveOp("MINACC2X", spec, subdim=False, uops_sha={})
    OPS.append(op)
    dve_ops.CUSTOM_DVE_SPECS[op.name] = op.spec
    dve_ops._SUB_OPCODE_FOR_NAME[op.name] = (
        dve_ops._CUSTOM_DVE_ROW_BASE + len(OPS) - 1)
    return op


def _emit_2x(nc, op, out, in0, in1):
    """InstCustomDveAnt with perf_max=1 (mirrors bass._custom_dve)."""
    from concourse import bass_isa, mybir
    from concourse.dve_ops import get_dve_sub_opcode
    v = nc.vector
    if op.name not in nc.m.ant_custom_dve_ops:
        nc.m.ant_custom_dve_ops = sorted({*nc.m.ant_custom_dve_ops, op.name})
    isa_opcode = nc.isa.Opcode[
        "NEURON_ISA_TPB_OPCODE_CUSTOM_DVE_ANT_"
        f"{bass_isa.CustomDveShape.TTSS.slot()}"].value
    ins = [v.lower_ap(in0, for_isa=True, opt=True),
           v.lower_ap(in1, for_isa=True, opt=True),
           mybir.ImmediateValue(dtype=mybir.dt.float32, value=60000.0),
           mybir.ImmediateValue(dtype=mybir.dt.float32, value=1023.0)]
    outs = [v.lower_ap(out, for_isa=True, opt=True)]
    return v.add_instruction(bass_isa.InstCustomDveAnt(
        name=nc.get_next_instruction_name(),
        op_name=op.name, rd1_en=True, subdim=0, imm2=0.0,
        shape=bass_isa.CustomDveShape.TTSS, row=get_dve_sub_opcode(op.name),
        isa_opcode=isa_opcode, ins=ins, outs=outs, perf_max=1))


def _build():
    import concourse.bass as bass
    import concourse.bacc as bacc
    import concourse.tile as tile
    from concourse import mybir

    MINACC, MININIT, MIN2X = _register_ops()
    NA = 2   # tiles consumed straight from PSUM by the 1x op per sample

    f32, f16, f8 = mybir.dt.float32, mybir.dt.float16, mybir.dt.float8e4
    AF = mybir.ActivationFunctionType
    DR = mybir.MatmulPerfMode.DoubleRow

    nc = bacc.Bacc("TRN2", target_bir_lowering=False, debug=False,
                   num_devices=NCORES)
    stat_d = nc.declare_dram_parameter("stat8", [BL, 128, NT, 2, 128], f8,
                                       isOutput=False)
    mov_d = nc.declare_dram_parameter("mov8", [BL, 128, 2, N], f8,
                                      isOutput=False)
    ch0_o = nc.declare_dram_parameter("ch0_part", [N, N], f16, isOutput=True)
    ch1_o = nc.declare_dram_parameter("ch1_part", [128, BL, NT], f16,
                                      isOutput=True)

    with tile.TileContext(nc) as tc:
        with (
            tc.tile_pool(name="stat", bufs=2) as statp,
            tc.tile_pool(name="mov", bufs=2) as movp,
            tc.tile_pool(name="z16p", bufs=4) as z16p,
            tc.tile_pool(name="persist", bufs=1) as perp,
            tc.tile_pool(name="nps", bufs=1, space=bass.MemorySpace.PSUM) as nps,
        ):
            acc = perp.tile([128, NT, N], f16, tag="acc")
            ch1z = perp.tile([128, BL, NT], f16, tag="ch1z")
            big = perp.tile([128, N], f16, tag="big")
            nc.gpsimd.memset(big[:], 60000.0)

            gt = [nps.tile([128, N], f32, tag=f"g{i}", name=f"g{i}")
                  for i in range(4)]

            for b in range(BL):
                # split loads so the first tiles' operands land early; b=0
                # fans out over three DMA queues to shorten the pipeline fill
                stat = statp.tile([128, NT, 2, 128], f8, tag="stat")
                mov = movp.tile([128, 2, N], f8, tag="mov")
                if b == 0:
                    nc.sync.dma_start(stat[:, 0:2], stat_d[b, :, 0:2])
                    nc.scalar.dma_start(mov[:, :, 0:256],
                                        mov_d[b, :, :, 0:256])
                    nc.sync.dma_start(mov[:, :, 256:512],
                                      mov_d[b, :, :, 256:512])
                    nc.gpsimd.dma_start(mov[:, :, 512:N],
                                        mov_d[b, :, :, 512:N])
                    nc.sync.dma_start(stat[:, 2:NT], stat_d[b, :, 2:NT])
                else:
                    nc.sync.dma_start(stat[:, 0:2], stat_d[b, :, 0:2])
                    nc.gpsimd.dma_start(mov[:, :, 0:512],
                                        mov_d[b, :, :, 0:512])
                    nc.sync.dma_start(stat[:, 2:NT], stat_d[b, :, 2:NT])
                    nc.gpsimd.dma_start(mov[:, :, 512:N],
                                        mov_d[b, :, :, 512:N])

                # A-tiles (0,1) consumed mid-stream so the ACT cast
                # pipeline builds inventory during the long 1x ops
                for seq, mt in enumerate((0, 2, 3, 1, 4, 5, 6, 7)):
                    g = gt[seq % 4]
                    for c in range(2):
                        nc.tensor.matmul(
                            g[:, c * 512:(c + 1) * 512],
                            stat[:, mt, :, :],
                            mov[:, :, c * 512:(c + 1) * 512],
                            start=True, stop=True, perf_mode=DR)
                    if mt < NA:
                        # PSUM-direct fused 1x consume
                        if b == 0:
                            nc.vector._custom_dve(
                                MININIT, out=acc[:, mt, :], in0=g[:],
                                s0=60000.0, s1=1023.0)
                        else:
                            nc.vector._custom_dve(
                                MINACC, out=acc[:, mt, :], in0=g[:],
                                in1=acc[:, mt, :], s0=60000.0, s1=1023.0)
                    else:
                        # ACT casts PSUM->fp16, then the 2X_1PORT op
                        z16 = z16p.tile([128, N], f16, tag="z16")
                        nc.scalar.activation(out=z16[:], in_=g[:],
                                             func=AF.Copy)
                        _emit_2x(nc, MIN2X, out=acc[:, mt, :], in0=z16[:],
                                 in1=(big[:] if b == 0
                                      else acc[:, mt, :]))
                    if b == BL - 1:
                        # acc[mt] final: stream it out under remaining
                        # compute, alternating rings; the last two tiles
                        # split so no single transfer tails past the end
                        # queue map tuned to issue order (0,2,3,1,4,5,..):
                        # slow gpsimd gets only the two earliest tiles,
                        # scalar (casts nearly done) takes mid/late ones
                        lo, hi = mt * 128, (mt + 1) * 128
                        ring = {0: nc.gpsimd, 2: nc.gpsimd, 3: nc.sync,
                                1: nc.scalar, 4: nc.sync, 5: nc.scalar}
                        if mt < NT - 2:
                            ring[mt].dma_start(ch0_o[lo:hi, :], acc[:, mt, :])
                        elif mt == NT - 2:
                            nc.scalar.dma_start(ch0_o[lo:hi, 0:512],
                                                acc[:, mt, 0:512])
                            nc.gpsimd.dma_start(ch0_o[lo:hi, 512:N],
                                                acc[:, mt, 512:N])
                        else:
                            nc.scalar.dma_start(ch0_o[lo:hi, 0:512],
                                                acc[:, mt, 0:512])
                            nc.sync.dma_start(ch0_o[lo:hi, 512:N],
                                              acc[:, mt, 512:N])
                # harvest this b's min_n d2 (scan cols) before b+1 overwrites;
                # two halves, so half 1 is done before b+1's first custom op
                nc.scalar.activation(out=ch1z[:, b, 0:4],
                                     in_=acc[:, 0:4, N - 1], func=AF.Copy)
                nc.scalar.activation(out=ch1z[:, b, 4:NT],
                                     in_=acc[:, 4:NT, N - 1], func=AF.Copy)
                nc.gpsimd.dma_start(ch1_o[:, b, :], ch1z[:, b, :])

    nc.compile()
    return nc


def _get_nc():
    if "nc" not in _CACHE:
        _CACHE["nc"] = _build()
    return _CACHE["nc"]


def _pack_core(pred_s, targ_s):
    """Build stat8/mov8 fp8 operands for one core's BL samples."""
    stat8 = np.zeros((BL, 128, NT, 2, 128), F8)
    mov8 = np.zeros((BL, 128, 2, N), F8)
    one8 = np.asarray(1.0, F8)
    for b in range(BL):
        T = targ_s[b]                    # [N, D]
        P = pred_s[b]
        tn = (T.astype(np.float64) ** 2).sum(-1).astype(np.float32)  # [N]
        pn = (P.astype(np.float64) ** 2).sum(-1).astype(np.float32)

        # 3-term fp8 residual cascades of tn / pn
        def casc(v):
            terms, rem = [], v.copy()
            for _ in range(3):
                t = np.asarray(rem, F8)
                terms.append(t)
                rem = rem - t.astype(np.float32)
            return terms

        tn_t, pn_t = casc(tn), casc(pn)

        Tt2 = np.asarray(-2.0 * T.T, F8)          # [d=128, m_global]
        stat8[b, :, :, 0, :] = Tt2.reshape(128, NT, 128)
        for j in range(3):
            stat8[b, j, :, 1, :] = one8                       # pn ones
            stat8[b, 3 + j, :, 1, :] = tn_t[j].reshape(NT, 128)

        mov8[b, :, 0, :] = np.asarray(P.T, F8)    # [d, n]
        for j in range(3):
            mov8[b, j, 1, :] = pn_t[j]
            mov8[b, 3 + j, 1, :] = one8
    return stat8, mov8


def run_device(pred, target, trace=False, **kw):
    from concourse.bass_utils import run_bass_kernel_spmd

    nc = _get_nc()
    ins = []
    for i in range(NCORES):
        sl = slice(i * BL, (i + 1) * BL)
        stat8, mov8 = _pack_core(pred[sl], target[sl])
        ins.append({"stat8": stat8, "mov8": mov8})
    return run_bass_kernel_spmd(nc, ins, list(range(NCORES)), trace=trace, **kw)


def kernel(pred, target):
    pred = np.ascontiguousarray(np.asarray(pred, dtype=np.float32))
    target = np.ascontiguousarray(np.asarray(target, dtype=np.float32))
    res = run_device(pred, target)
    rs = res.results

    # chamfer min over dim=0 (batch): cross-core elementwise min of acc
    d0 = rs[0]["ch0_part"].astype(np.float32)
    for r in rs[1:]:
        d0 = np.minimum(d0, r["ch0_part"].astype(np.float32))
    # col N-1 was overwritten by the scan output on device; recompute exact
    lastp = pred[:, N - 1, :]                              # [B, D]
    dlast = ((target.astype(np.float64)
              - lastp[:, None, :].astype(np.float64)) ** 2).sum(-1)  # [B, N]
    d0[:, N - 1] = dlast.min(axis=0)
    ch0 = np.sqrt(np.maximum(d0.astype(np.float64), 1e-12)).mean()

    # chamfer min over dim=1: scan cols, [core][p, b_local, mt] -> [B, N]
    ch1 = np.concatenate(
        [r["ch1_part"].astype(np.float64).transpose(1, 2, 0).reshape(BL, N)
         for r in rs], axis=0)                              # [B, N]
    ch1 = np.sqrt(np.maximum(ch1, 1e-12)).mean()

    mae = np.abs(pred.astype(np.float64) - target.astype(np.float64)).mean()

    p = np.sort(pred.reshape(B, -1), axis=1)
    g = np.sort(target.reshape(B, -1), axis=1)
    emd = np.abs(p - g).mean(axis=1, dtype=np.float64)

    return (mae + ch0 + ch1 + emd).astype(np.float32)


# revision 34
# speedup vs baseline: 1.0666x; 1.0185x over previous
"""Trainium2 Bass kernel for nn_CustomLoss_35940286333129.

loss[b] = mean|pred-target| (mae, scalar)
        + mean(min_n cdist[b,n,m]) + mean(min_b cdist[b,n,m])  (chamfer, scalar)
        + mean|sort(pred[b].ravel()) - sort(target[b].ravel())|  (emd, per-b)

Sharding: data-parallel over batch B=32 across 8 NeuronCores (4 samples each).

Device kernel (per local sample b):
  One fp8 DoubleRow matmul per 128-row tile computes the COMPLETE squared
  distance d2[m, n] = tn[m] + pn[n] - 2*T[m].P[n] directly in PSUM:
  the K=256 contraction carries -2*T^t x P^t in the first K-half and the
  norm biases in the second K-half (tn/pn shipped from the host as 3-term
  fp8 residual cascades against ones rows). 512 PE cycles per tile;
  no ones-matmul, no cast/transpose chains, no DRAM bounce.

  One fused custom DVE op consumes each PSUM tile in a single 1x pass:
      out = where(Idx == 1023, running_min(d2), min(d2, acc))
  so cols 0..1022 update the cross-sample elementwise min (chamfer min over
  dim=0) while col 1023 captures min_n d2 (chamfer min over dim=1), which
  ACT harvests per sample before the next overwrite. PSUM holds four exact
  [128,1024] tiles (no pad column), double-buffering the PE four deep.

Host: fp8 operand packing (transpose/cast/norm cascades) during sharding,
cross-core elementwise min + sqrt + means, the exact column n=1023 of the
chamfer dim-0 min (overwritten on-device by the scan output; 32x1024 dot
products in numpy), mae, and the exact per-sample EMD via np.sort (sort is
unsupported on trn2).
"""

import numpy as np
import ml_dtypes

F8 = ml_dtypes.float8_e4m3

B, N, D = 32, 1024, 128
NCORES = 8
BL = B // NCORES          # 4 local samples per core
NT = N // 128             # 8 row tiles

_CACHE = {}


def _register_ops():
    from concourse import dve_ops
    from concourse.dve_ops import DveOp, OPS, DveOpSpec
    from concourse.dve_spec import (Spec, Src0, Src1, C0, C1, scan, minn,
                                    select, eq, lower, AluOp, Idx)

    def _mk(name, body, ref, rd1):
        for op in OPS:
            if op.name == name:
                return op
        spec = Spec(body=body, reference=ref)
        shas = {}
        for ver in ("v3", "v4"):
            tmp = DveOpSpec(name=name, opcode=0, uops=lower(spec, ver=ver),
                            rd1_en=rd1)
            shas[ver] = tmp.sha(ver)
        op = DveOp(name, spec, subdim=False, uops_sha=shas)
        OPS.append(op)
        dve_ops.CUSTOM_DVE_SPECS[op.name] = op.spec
        dve_ops._SUB_OPCODE_FOR_NAME[op.name] = (
            dve_ops._CUSTOM_DVE_ROW_BASE + len(OPS) - 1)
        return op

    r = scan(AluOp.MIN, Src0, init=C0)

    def ref_acc(in0, in1, s0, s1, imm2):
        idx = np.arange(in0.shape[-1])
        state = np.minimum.accumulate(np.minimum(in0, s0), axis=-1)
        return np.where(idx == s1, state, np.minimum(in0, in1))

    def ref_init(in0, s0, s1, imm2):
        idx = np.arange(in0.shape[-1])
        state = np.minimum.accumulate(np.minimum(in0, s0), axis=-1)
        return np.where(idx == s1, state, in0)

    acc_op = _mk("MINACC_IDX", select(eq(Idx, C1), r, minn(Src0, Src1)),
                 ref_acc, True)
    init_op = _mk("MININIT_IDX", select(eq(Idx, C1), r, Src0), ref_init, False)
    return acc_op, init_op, _register_2x_op()


def _build_2x_uops():
    """Hand-built 2X_1PORT program (HW-verified bit-exact vs the 1x body).

    Trigger-sequenced, no datapath counter: uop0 inits the scan flop S to
    MAX_POS; uops 1-3 stream 255+255+1=511 pairs computing WR0_LO =
    min(z_lo, a_lo), WR0_HI = min(z_hi, a_hi), S = min(S, z_lo, z_hi);
    uop4 takes the final pair with WR0_HI = min(S, z_lo, z_hi) — i.e. the
    row min lands at element 1023, matching the 1x body. Hardcodes a
    1024-element row. Chain c carries lane c+1 (lane 0 reachable only at
    block 0, left empty)."""
    from concourse.dve_uop import (UopConfig, UopDpConfig, InpSel, OutPath,
                                   OutSel, AluInp, DelayInp, Trigger, AluOp)

    INP = [InpSel.ZERO, InpSel.SRC_0, InpSel.SRC_1, InpSel.SRC_0_HI,
           InpSel.SRC_1_HI, InpSel.MAX_POS, InpSel.ZERO, InpSel.ZERO]
    INP_EN = [0, 1, 1, 1, 1, 1, 0, 0]

    def dp_block(op, s0, s1, cap4=False, cap5=False, aoe=1):
        delay = [DelayInp.PREV_DELAY] * 7
        if cap4:
            delay[4] = DelayInp.PREV_ALU_OUT
        if cap5:
            delay[5] = DelayInp.PREV_ALU_OUT
        return UopDpConfig(op=op, alu_src0=s0, alu_src1=s1, delay=delay,
                           alu_out_enable=aoe,
                           delay_enable=[1, 1, 1, 1, 1, 1, 0])

    def stream_blocks():
        return [
            dp_block(AluOp.MIN, AluInp.PREV_DELAY_0, AluInp.PREV_DELAY_1),
            dp_block(AluOp.MIN, AluInp.PREV_DELAY_2, AluInp.PREV_DELAY_3,
                     cap5=True),
            dp_block(AluOp.MIN, AluInp.PREV_DELAY_0, AluInp.PREV_DELAY_2,
                     cap4=True),
            dp_block(AluOp.MIN, AluInp.CURR_ALU_OUT, AluInp.PREV_ALU_OUT),
            dp_block(AluOp.BYPASS, AluInp.PREV_ALU_OUT, AluInp.PREV_ALU_OUT),
            dp_block(AluOp.BYPASS, AluInp.PREV_ALU_OUT, AluInp.PREV_ALU_OUT),
            dp_block(AluOp.BYPASS, AluInp.PREV_ALU_OUT, AluInp.PREV_ALU_OUT),
            dp_block(AluOp.BYPASS, AluInp.PREV_ALU_OUT, AluInp.PREV_ALU_OUT),
        ]

    def init_blocks():
        bp = lambda: dp_block(AluOp.BYPASS, AluInp.PREV_DELAY_0,
                              AluInp.PREV_DELAY_0, aoe=0)
        blocks = [bp(), bp(), bp(),
                  dp_block(AluOp.BYPASS, AluInp.PREV_DELAY_4,
                           AluInp.PREV_DELAY_4)]
        for _ in range(4):
            blocks.append(dp_block(AluOp.BYPASS, AluInp.PREV_ALU_OUT,
                                   AluInp.PREV_ALU_OUT, aoe=0))
        return blocks

    def mk(blocks, out, out_en, req, rep, trig, nxt):
        return UopConfig(inp=list(INP), inp_enable=list(INP_EN),
                         out=out, out_enable=out_en,
                         require_inp0=req, require_inp1=req,
                         repeat_count=rep, trigger=trig, next_uop=nxt,
                         datapath_config=blocks)

    OUT_OFF = {OutPath.WR0_LO: OutSel.ALU_OUT, OutPath.WR0_HI: OutSel.ALU_OUT,
               OutPath.WR1_LO: OutSel.ALU_OUT, OutPath.WR1_HI: OutSel.ALU_OUT}
    EN_OFF = {OutPath.WR0_LO: 0, OutPath.WR0_HI: 0,
              OutPath.WR1_LO: 0, OutPath.WR1_HI: 0}
    OUT_STREAM = {OutPath.WR0_LO: OutSel.DELAY_5,
                  OutPath.WR0_HI: OutSel.DELAY_4,
                  OutPath.WR1_LO: OutSel.ALU_OUT,
                  OutPath.WR1_HI: OutSel.ALU_OUT}
    EN_RW = {OutPath.WR0_LO: 1, OutPath.WR0_HI: 1,
             OutPath.WR1_LO: 0, OutPath.WR1_HI: 0}
    OUT_LAST = {OutPath.WR0_LO: OutSel.DELAY_5, OutPath.WR0_HI: OutSel.ALU_OUT,
                OutPath.WR1_LO: OutSel.ALU_OUT, OutPath.WR1_HI: OutSel.ALU_OUT}

    T = Trigger
    return [
        mk(init_blocks(), OUT_OFF, EN_OFF, 0, 1, (T.COUNT, T.NONE, T.NONE),
           (1, 0, 0)),
        mk(stream_blocks(), OUT_STREAM, EN_RW, 1, 255,
           (T.COUNT, T.NONE, T.NONE), (2, 0, 0)),
        mk(stream_blocks(), OUT_STREAM, EN_RW, 1, 255,
           (T.COUNT, T.NONE, T.NONE), (3, 0, 0)),
        mk(stream_blocks(), OUT_STREAM, EN_RW, 1, 1,
           (T.COUNT, T.NONE, T.NONE), (4, 0, 0)),
        mk(stream_blocks(), OUT_LAST, EN_RW, 1, 0,
           (T.SRC_TENSOR_DONE, T.NONE, T.NONE), (0, 0, 0)),
    ]


def _register_2x_op():
    from concourse import dve_ops
    from concourse.dve_ops import DveOp, OPS, DveOpSpec, get_dve_sub_opcode
    from concourse.dve_spec import (Spec, Src0, Src1, C0, C1, scan, minn,
                                    select, eq, lower, AluOp, Idx)

    for op in OPS:
        if op.name == "MINACC2X":
            return op

    r = scan(AluOp.MIN, Src0, init=C0)
    body = select(eq(Idx, C1), r, minn(Src0, Src1))

    def ref(in0, in1, s0, s1, imm2):
        idx = np.arange(in0.shape[-1])
        state = np.minimum.accumulate(np.minimum(in0, s0), axis=-1)
        return np.where(idx == s1, state, np.minimum(in0, in1))

    spec = Spec(body=body, reference=ref)

    class PerfDveOp(DveOp):
        def compile(self, ver):
            key = ("MINACC2X", ver)
            if key in dve_ops._COMPILE_CACHE:
                return dve_ops._COMPILE_CACHE[key]
            import copy
            uops_1x = lower(spec, ver=ver)
            while len(uops_1x) < 5:   # pad REGULAR to the 2x state count
                pad = copy.deepcopy(uops_1x[-1])
                pad.next_uop = (0, 0, 0)
                uops_1x.append(pad)
            result = DveOpSpec(
                name="MINACC2X", opcode=get_dve_sub_opcode("MINACC2X"),
                uops=uops_1x, uops_2x=_build_2x_uops(), rd1_en=True,
                perf_max=1)
            dve_ops._COMPILE_CACHE[key] = result
            return result

    op = PerfDveOp("MINACC2X", spec, subdim=False, uops_sha={})
    OPS.append(op)
    dve_ops.CUSTOM_DVE_SPECS[op.name] = op.spec
    dve_ops._SUB_OPCODE_FOR_NAME[op.name] = (
        dve_ops._CUSTOM_DVE_ROW_BASE + len(OPS) - 1)
    return op


def _emit_2x(nc, op, out, in0, in1):
    """InstCustomDveAnt with perf_max=1 (mirrors bass._custom_dve)."""
    from concourse import bass_isa, mybir
    from concourse.dve_ops import get_dve_sub_opcode
    v = nc.vector
    if op.name not in nc.m.ant_custom_dve_ops:
        nc.m.ant_custom_dve_ops = sorted({*nc.m.ant_custom_dve_ops, op.name})
    isa_opcode = nc.isa.Opcode[
        "NEURON_ISA_TPB_OPCODE_CUSTOM_DVE_ANT_"
        f"{bass_isa.CustomDveShape.TTSS.slot()}"].value
    ins = [v.lower_ap(in0, for_isa=True, opt=True),
           v.lower_ap(in1, for_isa=True, opt=True),
           mybir.ImmediateValue(dtype=mybir.dt.float32, value=60000.0),
           mybir.ImmediateValue(dtype=mybir.dt.float32, value=1023.0)]
    outs = [v.lower_ap(out, for_isa=True, opt=True)]
    return v.add_instruction(bass_isa.InstCustomDveAnt(
        name=nc.get_next_instruction_name(),
        op_name=op.name, rd1_en=True, subdim=0, imm2=0.0,
        shape=bass_isa.CustomDveShape.TTSS, row=get_dve_sub_opcode(op.name),
        isa_opcode=isa_opcode, ins=ins, outs=outs, perf_max=1))


def _build():
    import concourse.bass as bass
    import concourse.bacc as bacc
    import concourse.tile as tile
    from concourse import mybir

    MINACC, MININIT, MIN2X = _register_ops()
    NA = 2   # tiles consumed straight from PSUM by the 1x op per sample

    f32, f16, f8 = mybir.dt.float32, mybir.dt.float16, mybir.dt.float8e4
    AF = mybir.ActivationFunctionType
    DR = mybir.MatmulPerfMode.DoubleRow

    nc = bacc.Bacc("TRN2", target_bir_lowering=False, debug=False,
                   num_devices=NCORES)
    stat_d = nc.declare_dram_parameter("stat8", [BL, 128, NT, 2, 128], f8,
                                       isOutput=False)
    mov_d = nc.declare_dram_parameter("mov8", [BL, 128, 2, N], f8,
                                      isOutput=False)
    ch0_o = nc.declare_dram_parameter("ch0_part", [N, N], f16, isOutput=True)
    ch1_o = nc.declare_dram_parameter("ch1_part", [128, BL, NT], f16,
                                      isOutput=True)

    with tile.TileContext(nc) as tc:
        with (
            tc.tile_pool(name="stat", bufs=2) as statp,
            tc.tile_pool(name="mov", bufs=2) as movp,
            tc.tile_pool(name="z16p", bufs=4) as z16p,
            tc.tile_pool(name="persist", bufs=1) as perp,
            tc.tile_pool(name="nps", bufs=1, space=bass.MemorySpace.PSUM) as nps,
        ):
            acc = perp.tile([128, NT, N], f16, tag="acc")
            ch1z = perp.tile([128, BL, NT], f16, tag="ch1z")
            big = perp.tile([128, N], f16, tag="big")
            nc.gpsimd.memset(big[:], 60000.0)

            gt = [nps.tile([128, N], f32, tag=f"g{i}", name=f"g{i}")
                  for i in range(4)]

            for b in range(BL):
                # split loads so the first tiles' operands land early; b=0
                # fans out over three DMA queues to shorten the pipeline fill
                stat = statp.tile([128, NT, 2, 128], f8, tag="stat")
                mov = movp.tile([128, 2, N], f8, tag="mov")
                if b == 0:
                    nc.sync.dma_start(stat[:, 0:2], stat_d[b, :, 0:2])
                    nc.scalar.dma_start(mov[:, :, 0:512],
                                        mov_d[b, :, :, 0:512])
                    nc.gpsimd.dma_start(mov[:, :, 512:N],
                                        mov_d[b, :, :, 512:N])
                    nc.sync.dma_start(stat[:, 2:NT], stat_d[b, :, 2:NT])
                else:
                    nc.sync.dma_start(stat[:, 0:2], stat_d[b, :, 0:2])
                    nc.gpsimd.dma_start(mov[:, :, 0:512],
                                        mov_d[b, :, :, 0:512])
                    nc.sync.dma_start(stat[:, 2:NT], stat_d[b, :, 2:NT])
                    nc.gpsimd.dma_start(mov[:, :, 512:N],
                                        mov_d[b, :, :, 512:N])

                # A-tiles (0,1) consumed mid-stream so the ACT cast
                # pipeline builds inventory during the long 1x ops
                for seq, mt in enumerate((0, 2, 3, 1, 4, 5, 6, 7)):
                    g = gt[seq % 4]
                    for c in range(2):
                        nc.tensor.matmul(
                            g[:, c * 512:(c + 1) * 512],
                            stat[:, mt, :, :],
                            mov[:, :, c * 512:(c + 1) * 512],
                            start=True, stop=True, perf_mode=DR)
                    if mt < NA:
                        # PSUM-direct fused 1x consume
                        if b == 0:
                            nc.vector._custom_dve(
                                MININIT, out=acc[:, mt, :], in0=g[:],
                                s0=60000.0, s1=1023.0)
                        else:
                            nc.vector._custom_dve(
                                MINACC, out=acc[:, mt, :], in0=g[:],
                                in1=acc[:, mt, :], s0=60000.0, s1=1023.0)
                    else:
                        # ACT casts PSUM->fp16, then the 2X_1PORT op
                        z16 = z16p.tile([128, N], f16, tag="z16")
                        nc.scalar.activation(out=z16[:], in_=g[:],
                                             func=AF.Copy)
                        _emit_2x(nc, MIN2X, out=acc[:, mt, :], in0=z16[:],
                                 in1=(big[:] if b == 0
                                      else acc[:, mt, :]))
                    if b == BL - 1:
                        # acc[mt] final: stream it out under remaining
                        # compute, alternating rings; the last two tiles
                        # split so no single transfer tails past the end
                        lo, hi = mt * 128, (mt + 1) * 128
                        if mt < NT - 2:
                            ring = nc.gpsimd if mt % 2 == 0 else nc.sync
                            ring.dma_start(ch0_o[lo:hi, :], acc[:, mt, :])
                        else:
                            # scalar queue is idle by now; keep gpsimd free
                            # to drain its earlier tiles
                            nc.scalar.dma_start(ch0_o[lo:hi, 0:512],
                                                acc[:, mt, 0:512])
                            nc.sync.dma_start(ch0_o[lo:hi, 512:N],
                                              acc[:, mt, 512:N])
                # harvest this b's min_n d2 (scan cols) before b+1 overwrites;
                # two halves, so half 1 is done before b+1's first custom op
                nc.scalar.activation(out=ch1z[:, b, 0:4],
                                     in_=acc[:, 0:4, N - 1], func=AF.Copy)
                nc.scalar.activation(out=ch1z[:, b, 4:NT],
                                     in_=acc[:, 4:NT, N - 1], func=AF.Copy)
                nc.gpsimd.dma_start(ch1_o[:, b, :], ch1z[:, b, :])

    nc.compile()
    return nc


def _get_nc():
    if "nc" not in _CACHE:
        _CACHE["nc"] = _build()
    return _CACHE["nc"]


def _pack_core(pred_s, targ_s):
    """Build stat8/mov8 fp8 operands for one core's BL samples."""
    stat8 = np.zeros((BL, 128, NT, 2, 128), F8)
    mov8 = np.zeros((BL, 128, 2, N), F8)
    one8 = np.asarray(1.0, F8)
    for b in range(BL):
        T = targ_s[b]                    # [N, D]
        P = pred_s[b]
        tn = (T.astype(np.float64) ** 2).sum(-1).astype(np.float32)  # [N]
        pn = (P.astype(np.float64) ** 2).sum(-1).astype(np.float32)

        # 3-term fp8 residual cascades of tn / pn
        def casc(v):
            terms, rem = [], v.copy()
            for _ in range(3):
                t = np.asarray(rem, F8)
                terms.append(t)
                rem = rem - t.astype(np.float32)
            return terms

        tn_t, pn_t = casc(tn), casc(pn)

        Tt2 = np.asarray(-2.0 * T.T, F8)          # [d=128, m_global]
        stat8[b, :, :, 0, :] = Tt2.reshape(128, NT, 128)
        for j in range(3):
            stat8[b, j, :, 1, :] = one8                       # pn ones
            stat8[b, 3 + j, :, 1, :] = tn_t[j].reshape(NT, 128)

        mov8[b, :, 0, :] = np.asarray(P.T, F8)    # [d, n]
        for j in range(3):
            mov8[b, j, 1, :] = pn_t[j]
            mov8[b, 3 + j, 1, :] = one8
    return stat8, mov8


def run_device(pred, target, trace=False, **kw):
    from concourse.bass_utils import run_bass_kernel_spmd

    nc = _get_nc()
    ins = []
    for i in range(NCORES):
        sl = slice(i * BL, (i + 1) * BL)
        stat8, mov8 = _pack_core(pred[sl], target[sl])
        ins.append({"stat8": stat8, "mov8": mov8})
    return run_bass_kernel_spmd(nc, ins, list(range(NCORES)), trace=trace, **kw)


def kernel(pred, target):
    pred = np.ascontiguousarray(np.asarray(pred, dtype=np.float32))
    target = np.ascontiguousarray(np.asarray(target, dtype=np.float32))
    res = run_device(pred, target)
    rs = res.results

    # chamfer min over dim=0 (batch): cross-core elementwise min of acc
    d0 = rs[0]["ch0_part"].astype(np.float32)
    for r in rs[1:]:
        d0 = np.minimum(d0, r["ch0_part"].astype(np.float32))
    # col N-1 was overwritten by the scan output on device; recompute exact
    lastp = pred[:, N - 1, :]                              # [B, D]
    dlast = ((target.astype(np.float64)
              - lastp[:, None, :].astype(np.float64)) ** 2).sum(-1)  # [B, N]
    d0[:, N - 1] = dlast.min(axis=0)
    ch0 = np.sqrt(np.maximum(d0.astype(np.float64), 1e-12)).mean()

    # chamfer min over dim=1: scan cols, [core][p, b_local, mt] -> [B, N]
    ch1 = np.concatenate(
        [r["ch1_part"].astype(np.float64).transpose(1, 2, 0).reshape(BL, N)
         for r in rs], axis=0)                              # [B, N]
    ch1 = np.sqrt(np.maximum(ch1, 1e-12)).mean()

    mae = np.abs(pred.astype(np.float64) - target.astype(np.float64)).mean()

    p = np.sort(pred.reshape(B, -1), axis=1)
    g = np.sort(target.reshape(B, -1), axis=1)
    emd = np.abs(p - g).mean(axis=1, dtype=np.float64)

    return (mae + ch0 + ch1 + emd).astype(np.float32)
